# revision 2
# baseline (speedup 1.0000x reference)
"""Trainium2 Bass kernel for nn_BigramLanguageModel (dense transformer block).

Reference computation (B=2, T=2048, E=1024, V=32000):
    x      = emb_table[X] + pos_table                       # [B,T,E]
    k,q,v  = x@Wk, x@Wq, x@Wv                               # [B,T,E]
    w      = (q @ k^T) / sqrt(E), causal mask (tril)        # [B,T,T]
    w      = softmax(w, axis=1)          # QUIRK: over the *query* axis
    out    = w @ v                                          # [B,T,E]
    logits = out @ Wro + bro                                # [B,T,V]

Sharding: 8 cores = 2 (batch) x 4 (vocab slices of 8000 for the readout
matmul, which dominates the FLOPs).  Each core runs the full attention for
its batch plus the readout for its vocab slice.

Device-side layout trick: scores are computed transposed, wT[k,q], so the
softmax-over-q runs along the free axis.  The softmax denominator depends
only on k, so it is folded into V (V' = V/denom[k]) and the attention
output is produced directly in outT[e,q] layout, which is exactly the lhsT
layout the readout matmul wants.  Causal masking uses the block structure:
chunks with q_end <= k0 are never computed nor read; only the single
diagonal 512-chunk per k-tile needs an additive staircase mask.

All matmul operands are bf16 (full PE rate), accumulation fp32 in PSUM.
"""

import sys

if "/opt/trn_rl_repo" not in sys.path:
    sys.path.insert(0, "/opt/trn_rl_repo")

import numpy as np
import ml_dtypes

import concourse.bass as bass
import concourse.tile as tile
from concourse import bacc, mybir
from concourse.bass_utils import run_bass_kernel_spmd

P = 128
B, T, E, VOC = 2, 2048, 1024, 32000
VSPLIT = 4                # vocab splits per batch group
VS = VOC // VSPLIT        # 8000 vocab columns per core
NE = E // P               # 8 embedding partition-tiles
NT = T // P               # 16 token partition-tiles
QCH = 512                 # q chunk width (scores / AV / proj free dim)
NQC = T // QCH            # 4
VCH = 500                 # vocab chunk width (<=512 psum bank, 8000 = 16*500)
NVC = VS // VCH           # 16
SCALE = 1.0 / 32.0        # 1/sqrt(E)
MASK_VAL = -960000.0      # additive pre-scale mask; /32 -> -30000 -> exp = 0

BF16 = mybir.dt.bfloat16
F32 = mybir.dt.float32

_CACHE: dict = {}


def _build_program():
    nc = bacc.Bacc("TRN2", target_bir_lowering=False, debug=False, num_devices=8)

    xT_d = nc.dram_tensor("xT", [NE, P, T], BF16, kind="ExternalInput").ap()
    wk_d = nc.dram_tensor("wk", [NE, P, E], BF16, kind="ExternalInput").ap()
    wq_d = nc.dram_tensor("wq", [NE, P, E], BF16, kind="ExternalInput").ap()
    wv_d = nc.dram_tensor("wv", [NE, P, E], BF16, kind="ExternalInput").ap()
    wro_d = nc.dram_tensor("wro", [NE, P, VS], BF16, kind="ExternalInput").ap()
    mask_d = nc.dram_tensor("mask", [P, NQC, QCH], F32, kind="ExternalInput").ap()
    out_d = nc.dram_tensor("logits", [NT, P, VS], F32, kind="ExternalOutput").ap()

    Exp = mybir.ActivationFunctionType.Exp

    with tile.TileContext(nc) as tc:
        from contextlib import ExitStack

        with ExitStack() as root:
            misc = root.enter_context(tc.tile_pool(name="misc", bufs=1))
            psum = root.enter_context(tc.tile_pool(name="psum", bufs=6, space="PSUM"))
            stage = root.enter_context(tc.tile_pool(name="stage", bufs=2))

            mask_t = misc.tile([P, NQC, QCH], F32, tag="mask", name="mask_t")
            nc.sync.dma_start(mask_t[:], mask_d[:])
            parts_t = misc.tile([P, NT, NQC], F32, tag="parts", name="parts_t")
            denom_t = misc.tile([P, NT], F32, tag="denom", name="denom_t")
            recip_t = misc.tile([P, NT], F32, tag="recip", name="recip_t")

            # ---- pools with phase-limited lifetimes --------------------
            pv = root.enter_context(tc.tile_pool(name="pv", bufs=1))
            v_t = [pv.tile([P, E], BF16, tag=f"v{i}", name=f"v{i}") for i in range(NT)]

            pkq = ExitStack()
            kq_pool = pkq.enter_context(tc.tile_pool(name="pkq", bufs=1))
            kT_t = [kq_pool.tile([P, T], BF16, tag=f"kT{i}", name=f"kT{i}") for i in range(NE)]
            qT_t = [kq_pool.tile([P, T], BF16, tag=f"qT{i}", name=f"qT{i}") for i in range(NE)]

            p1 = ExitStack()
            x_pool = p1.enter_context(tc.tile_pool(name="px", bufs=1))
            w_pool = p1.enter_context(tc.tile_pool(name="pw", bufs=2))

            # ================= phase 1: projections =====================
            xT_t = [x_pool.tile([P, T], BF16, tag=f"xT{i}", name=f"xT{i}") for i in range(NE)]
            for e in range(NE):
                nc.sync.dma_start(xT_t[e][:], xT_d[e])

            wk_t = w_pool.tile([P, NE, E], BF16, tag="w", name="wk_t")
            for e in range(NE):
                nc.sync.dma_start(wk_t[:, e, :], wk_d[e])
            wq_t = w_pool.tile([P, NE, E], BF16, tag="w", name="wq_t")
            for e in range(NE):
                nc.sync.dma_start(wq_t[:, e, :], wq_d[e])

            # kT[eo, t] = sum_e Wk[e, eo] * xT[e, t]
            for eo in range(NE):
                for tch in range(NQC):
                    ps = psum.tile([P, QCH], F32, tag="mm", name=f"ps_k{eo}_{tch}")
                    for e in range(NE):
                        nc.tensor.matmul(
                            ps[:],
                            wk_t[:, e, eo * P:(eo + 1) * P],
                            xT_t[e][:, tch * QCH:(tch + 1) * QCH],
                            start=(e == 0), stop=(e == NE - 1),
                        )
                    nc.scalar.copy(kT_t[eo][:, tch * QCH:(tch + 1) * QCH], ps[:])

            # qT — same shape; wv loads overlap qT compute
            wv_t = w_pool.tile([P, NE, E], BF16, tag="w", name="wv_t")
            for e in range(NE):
                nc.sync.dma_start(wv_t[:, e, :], wv_d[e])

            for eo in range(NE):
                for tch in range(NQC):
                    ps = psum.tile([P, QCH], F32, tag="mm", name=f"ps_q{eo}_{tch}")
                    for e in range(NE):
                        nc.tensor.matmul(
                            ps[:],
                            wq_t[:, e, eo * P:(eo + 1) * P],
                            xT_t[e][:, tch * QCH:(tch + 1) * QCH],
                            start=(e == 0), stop=(e == NE - 1),
                        )
                    nc.scalar.copy(qT_t[eo][:, tch * QCH:(tch + 1) * QCH], ps[:])

            # V[t, e] = sum_e' xT[e', t]^T * Wv[e', e]
            for tb in range(NT):
                for ec in range(E // QCH):
                    ps = psum.tile([P, QCH], F32, tag="mm", name=f"ps_v{tb}_{ec}")
                    for e in range(NE):
                        nc.tensor.matmul(
                            ps[:],
                            xT_t[e][:, tb * P:(tb + 1) * P],
                            wv_t[:, e, ec * QCH:(ec + 1) * QCH],
                            start=(e == 0), stop=(e == NE - 1),
                        )
                    nc.scalar.copy(v_t[tb][:, ec * QCH:(ec + 1) * QCH], ps[:])

            p1.close()  # frees xT + W pools

            # ============ phase 2: scores + softmax-over-q ==============
            # wT lives on the "right" SBUF stack so pkq (left) can be
            # released before it — pool releases must be LIFO per side.
            p2 = ExitStack()
            wT_pool = p2.enter_context(tc.tile_pool(name="pwT", bufs=1, side="right"))
            wT_t = [wT_pool.tile([P, T], BF16, tag=f"wT{i}", name=f"wT{i}") for i in range(NT)]

            for kt in range(NT):
                qcd = kt // 4  # diagonal q-chunk index for this k-tile
                for qc in range(qcd, NQC):
                    ps = psum.tile([P, QCH], F32, tag="mm", name=f"ps_s{kt}_{qc}")
                    for e in range(NE):
                        nc.tensor.matmul(
                            ps[:],
                            kT_t[e][:, kt * P:(kt + 1) * P],
                            qT_t[e][:, qc * QCH:(qc + 1) * QCH],
                            start=(e == 0), stop=(e == NE - 1),
                        )
                    wslice = wT_t[kt][:, qc * QCH:(qc + 1) * QCH]
                    acc = parts_t[:, kt, qc:qc + 1]
                    if qc == qcd:
                        dv = kt % 4
                        stg = stage.tile([P, QCH], F32, tag="stg", name=f"stg{kt}")
                        nc.vector.tensor_add(stg[:], ps[:], mask_t[:, dv, :])
                        nc.scalar.activation(wslice, stg[:], Exp, bias=0.0,
                                             scale=SCALE, accum_out=acc)
                    else:
                        nc.scalar.activation(wslice, ps[:], Exp, bias=0.0,
                                             scale=SCALE, accum_out=acc)
                # denom over computed chunks; fold 1/denom into V tile kt
                nc.vector.reduce_sum(denom_t[:, kt:kt + 1], parts_t[:, kt, qcd:NQC],
                                     axis=mybir.AxisListType.X)
                nc.vector.reciprocal(recip_t[:, kt:kt + 1], denom_t[:, kt:kt + 1])
                nc.vector.tensor_scalar_mul(v_t[kt][:], v_t[kt][:], recip_t[:, kt:kt + 1])

            pkq.close()  # frees kT/qT

            # ============ phase 3: outT[e,q] = V'^T @ wT ================
            p3 = ExitStack()
            outT_pool = p3.enter_context(tc.tile_pool(name="poutT", bufs=1))
            outT_t = [outT_pool.tile([P, T], BF16, tag=f"oT{i}", name=f"oT{i}") for i in range(NE)]

            for eb in range(NE):
                for qc in range(NQC):
                    kts = list(range(0, (qc + 1) * 4))  # k-tiles with k0 < q_end
                    ps = psum.tile([P, QCH], F32, tag="mm", name=f"ps_o{eb}_{qc}")
                    for i, kt in enumerate(kts):
                        nc.tensor.matmul(
                            ps[:],
                            v_t[kt][:, eb * P:(eb + 1) * P],
                            wT_t[kt][:, qc * QCH:(qc + 1) * QCH],
                            start=(i == 0), stop=(i == len(kts) - 1),
                        )
                    nc.scalar.copy(outT_t[eb][:, qc * QCH:(qc + 1) * QCH], ps[:])

            p2.close()  # frees wT

            # ============ phase 4: readout ==============================
            p4 = ExitStack()
            ro_pool = p4.enter_context(tc.tile_pool(name="pro", bufs=2))
            ostg_pool = p4.enter_context(tc.tile_pool(name="postg", bufs=4))

            for vc in range(NVC):
                wro_t = ro_pool.tile([P, NE, VCH], BF16, tag="wro", name=f"wro{vc}")
                for e in range(NE):
                    nc.sync.dma_start(wro_t[:, e, :], wro_d[e, :, vc * VCH:(vc + 1) * VCH])
                for tb in range(NT):
                    ps = psum.tile([P, VCH], F32, tag="mm", name=f"ps_r{vc}_{tb}")
                    for e in range(NE):
                        nc.tensor.matmul(
                            ps[:],
                            outT_t[e][:, tb * P:(tb + 1) * P],
                            wro_t[:, e, :],
                            start=(e == 0), stop=(e == NE - 1),
                        )
                    stg = ostg_pool.tile([P, VCH], F32, tag="ostg", name=f"ostg{vc}_{tb}")
                    if tb % 2 == 0:
                        nc.vector.tensor_copy(stg[:], ps[:])
                    else:
                        nc.scalar.copy(stg[:], ps[:])
                    nc.sync.dma_start(out_d[tb, :, vc * VCH:(vc + 1) * VCH], stg[:])

            p4.close()
            p3.close()

    nc.compile()
    return nc


def _get_nc():
    if "nc" not in _CACHE:
        _CACHE["nc"] = _build_program()
    return _CACHE["nc"]


def _make_in_maps(X, emb_table, pos_table, Wk, Wq, Wv, Wro):
    bf = ml_dtypes.bfloat16
    X = np.asarray(X)
    emb_table = np.asarray(emb_table, np.float32)
    pos_table = np.asarray(pos_table, np.float32)

    # host-side embedding gather + positional add (0.03% of model FLOPs)
    x = emb_table[X] + pos_table[None, :, :]            # [B, T, E] f32

    wk = np.ascontiguousarray(np.asarray(Wk, np.float32).reshape(NE, P, E)).astype(bf)
    wq = np.ascontiguousarray(np.asarray(Wq, np.float32).reshape(NE, P, E)).astype(bf)
    wv = np.ascontiguousarray(np.asarray(Wv, np.float32).reshape(NE, P, E)).astype(bf)

    Wro = np.asarray(Wro, np.float32)
    wro_s = []
    for s in range(VSPLIT):
        sl = Wro[:, s * VS:(s + 1) * VS].reshape(NE, P, VS)
        wro_s.append(np.ascontiguousarray(sl).astype(bf))

    xT_b = []
    for b in range(B):
        xt = np.ascontiguousarray(x[b].T).reshape(NE, P, T).astype(bf)
        xT_b.append(xt)

    # staircase masks for the diagonal chunk; variant dv = (k0 mod 512)/128
    p_idx = np.arange(P)[:, None]
    c_idx = np.arange(QCH)[None, :]
    mask = np.zeros((P, NQC, QCH), np.float32)
    for dv in range(NQC):
        mask[:, dv, :] = np.where(c_idx < dv * P + p_idx, MASK_VAL, 0.0)

    in_maps = []
    for c in range(8):
        b, s = divmod(c, VSPLIT)
        in_maps.append({
            "xT": xT_b[b],
            "wk": wk, "wq": wq, "wv": wv,
            "wro": wro_s[s],
            "mask": mask,
        })
    return in_maps


def run_on_device(in_maps, trace=False, **kw):
    nc = _get_nc()
    return run_bass_kernel_spmd(nc, in_maps, core_ids=list(range(8)), trace=trace, **kw)


def kernel(X, emb_table, pos_table, Wk, Wq, Wv, Wro, bro):
    in_maps = _make_in_maps(X, emb_table, pos_table, Wk, Wq, Wv, Wro)
    _CACHE["in_maps"] = in_maps

    res = run_on_device(in_maps, trace=False)
    _CACHE["last_results"] = res

    logits = np.empty((B, T, VOC), np.float32)
    for c in range(8):
        b, s = divmod(c, VSPLIT)
        logits[b, :, s * VS:(s + 1) * VS] = res.results[c]["logits"].reshape(T, VS)

    bro = np.asarray(bro, np.float32)
    if np.any(bro):
        logits += bro
    return logits
